# revision 16
# baseline (speedup 1.0000x reference)
"""DyBEM layer (histogram binning + embedding sum + linear) on 8 trn2 cores.

Math reduction
--------------
ref: xmin/xmax per column n over the batch; u = (x-xmin)/(xmax-xmin+eps);
     bins = cumsum(softmax(bin_logits)); idx = clip(searchsorted(bins, u), 0, 9)
     out  = einsum('bne,fe->bf', embed[idx], W) + IN_DIM * b

Let EW = embed @ W.T (shape [10,64]).  With g[b,k] = #{n : u[b,n] > bins[k]}
(k = 0..8):

  out[b] = IN_DIM*(EW[0] + b) + sum_k g[b,k] * (EW[k+1] - EW[k])

and u[b,n] > bins[k]  <=>  x[b,n] > T[k,n] := xmin[n] + bins[k]*(range[n]+eps).

Per core (batch shard of 4096 rows): local per-column min/max, AllGather the
8 cores' (min, -max) vectors, local reduce, then 9 indicator passes feeding a
PE contraction with the tiny D[k] = EW[k+1]-EW[k] rows.  The base row
IN_DIM*(EW[0]+b) is added as a per-partition bias during PSUM eviction.

Layout: x loads natural as [128, 32*64] (8KB contiguous per partition; SBUF
partition p holds batch rows p*32+t).  Each [128,128] column pair (t=2j,2j+1)
is PE-transposed into u [128=(par,n), 16*128] where column j*128+p holds batch
row b = p*32 + 2j + par split by t-parity across partition halves.  A single
per-partition-scalar is_gt (DVE 2x mode) then evaluates one bin for all 4096
rows, and matmuls with block-diagonal D tiles [128=(par,n), 128=(par,f)] keep
the parities separate, producing out.T [128=(par,f), (j,p)] in PSUM.  The host
undoes the (p,j,par) column permutation after gathering.
"""

import numpy as np

import concourse.bass as bass
import concourse.mybir as mybir
import concourse.tile as tile
from concourse import bacc, bass_utils
from concourse.masks import make_identity

F32 = mybir.dt.float32
F32R = mybir.dt.float32r
ALU = mybir.AluOpType
AX = mybir.AxisListType
ACT = mybir.ActivationFunctionType

B_FULL, IN_DIM, NUM_BINS, EMBED_DIM = 32768, 64, 10, 64
N_CORES = 8
B_C = B_FULL // N_CORES          # 4096 rows per core
EPS = 1e-6
P = 128
T_ALL = B_C // P                 # 32 row-groups (t index)
N_CHUNKS = 4                     # x load chunks
T_CHUNK = T_ALL // N_CHUNKS      # 8 t's per chunk
NTHR = NUM_BINS - 1              # 9 real thresholds
UCOLS = B_C // 2                 # 2048 u columns (2 rows per column)
MM_N = 512                       # matmul moving free size (one PSUM bank)


def _trace_kernel(tc, io, tag=""):
    nc = tc.nc
    x_d, bl_d, emb_d, w_d, b_d, out_d = io

    with (
        tc.tile_pool(name=f"const{tag}", bufs=1) as cpool,
        tc.tile_pool(name=f"ind{tag}", bufs=3) as ipool,
        tc.tile_pool(name=f"outs{tag}", bufs=2) as opool,
        tc.tile_pool(name=f"tp_psum{tag}", bufs=2, space="PSUM") as tp_psum,
        tc.tile_pool(name=f"out_psum{tag}", bufs=4, space="PSUM") as out_psum,
        tc.tile_pool(name=f"mc_psum{tag}", bufs=2, space="PSUM") as mc_psum,
        tc.tile_pool(name=f"dram{tag}", bufs=1, space="DRAM") as dpool,
    ):
        # ---------------- P0: parameters (independent of x) ----------------
        ident = cpool.tile([P, P], F32)
        make_identity(nc, ident[:])

        # bins = cumsum(softmax(bin_logits))
        bl_row = cpool.tile([1, NUM_BINS], F32)
        nc.sync.dma_start(bl_row[:], bl_d.unsqueeze(0))
        e_row = cpool.tile([1, NUM_BINS], F32)
        nc.scalar.activation(e_row[:], bl_row[:], ACT.Exp)
        ssum = cpool.tile([1, 1], F32)
        nc.vector.tensor_reduce(ssum[:], e_row[:], AX.X, ALU.add)
        rsum = cpool.tile([1, 1], F32)
        nc.vector.reciprocal(rsum[:], ssum[:])
        prob_a = cpool.tile([1, NUM_BINS], F32)
        nc.vector.tensor_scalar(prob_a[:], e_row[:], rsum[:, 0:1], None, ALU.mult)
        prob_b = cpool.tile([1, NUM_BINS], F32)
        cur, nxt = prob_a, prob_b
        for sh in (1, 2, 4, 8):
            nc.vector.tensor_copy(nxt[:, 0:sh], cur[:, 0:sh])
            nc.vector.tensor_tensor(
                nxt[:, sh:NUM_BINS], cur[:, sh:NUM_BINS], cur[:, 0 : NUM_BINS - sh],
                ALU.add,
            )
            cur, nxt = nxt, cur
        bins_d = dpool.tile([1, NUM_BINS], F32)
        nc.sync.dma_start(bins_d[:], cur[:])

        # bins broadcast to every partition: [128, 10]
        bins_bc = cpool.tile([P, NUM_BINS], F32)
        nc.gpsimd.dma_start(
            bins_bc[:],
            bins_d[:].broadcast_to([P, NUM_BINS]),
        )

        # EW = embed @ W.T ; D10 rows 0..8 = EW[k+1]-EW[k], row 9 = 64*(EW[0]+b)
        emb_s = cpool.tile([NUM_BINS, EMBED_DIM], F32)
        nc.sync.dma_start(emb_s[:], emb_d)
        w_s = cpool.tile([EMBED_DIM, EMBED_DIM], F32)
        nc.sync.dma_start(w_s[:], w_d)

        ps_embT = mc_psum.tile([EMBED_DIM, NUM_BINS], F32, tag="mc")
        nc.tensor.transpose(ps_embT[:], emb_s[:], ident[0:NUM_BINS, 0:NUM_BINS])
        embT_s = cpool.tile([EMBED_DIM, NUM_BINS], F32)
        nc.scalar.activation(embT_s[:], ps_embT[:], ACT.Copy)

        ps_wt = mc_psum.tile([EMBED_DIM, EMBED_DIM], F32, tag="mc")
        nc.tensor.transpose(ps_wt[:], w_s[:], ident[0:EMBED_DIM, 0:EMBED_DIM])
        wt_s = cpool.tile([EMBED_DIM, EMBED_DIM], F32)
        nc.scalar.activation(wt_s[:], ps_wt[:], ACT.Copy)

        ps_ew = mc_psum.tile([NUM_BINS, EMBED_DIM], F32, tag="mc")
        nc.tensor.matmul(ps_ew[:], embT_s[:], wt_s[:], start=True, stop=True)
        ew_ext = cpool.tile([NUM_BINS + 1, EMBED_DIM], F32)
        nc.scalar.activation(ew_ext[0:NUM_BINS, :], ps_ew[:], ACT.Copy)
        nc.sync.dma_start(ew_ext[NUM_BINS : NUM_BINS + 1, :], b_d.unsqueeze(0))

        # MT [11, 10]: D10 = MT.T @ ew_ext (bidiagonal diffs; col 9 = 64*(row0+rowb))
        mt = cpool.tile([NUM_BINS + 1, NUM_BINS], F32)
        nc.gpsimd.memset(mt[:], 0.0)
        nc.gpsimd.affine_select(
            out=mt[:, 0:NTHR], in_=mt[:, 0:NTHR], compare_op=ALU.not_equal,
            fill=-1.0, base=0, pattern=[[-1, NTHR]], channel_multiplier=1,
        )
        nc.gpsimd.affine_select(
            out=mt[:, 0:NTHR], in_=mt[:, 0:NTHR], compare_op=ALU.not_equal,
            fill=1.0, base=-1, pattern=[[-1, NTHR]], channel_multiplier=1,
        )
        nc.gpsimd.affine_select(
            out=mt[:, NTHR : NTHR + 1], in_=mt[:, NTHR : NTHR + 1],
            compare_op=ALU.not_equal, fill=float(IN_DIM),
            base=0, pattern=[[-1, 1]], channel_multiplier=1,
        )
        nc.gpsimd.affine_select(
            out=mt[:, NTHR : NTHR + 1], in_=mt[:, NTHR : NTHR + 1],
            compare_op=ALU.not_equal, fill=float(IN_DIM),
            base=-NUM_BINS, pattern=[[-1, 1]], channel_multiplier=1,
        )
        ps_d10 = mc_psum.tile([NUM_BINS, EMBED_DIM], F32, tag="mc")
        nc.tensor.matmul(ps_d10[:], mt[:], ew_ext[:], start=True, stop=True)
        d10_s = cpool.tile([NUM_BINS, EMBED_DIM], F32)
        nc.scalar.activation(d10_s[:], ps_d10[:], ACT.Copy)
        d10_d = dpool.tile([NUM_BINS, EMBED_DIM], F32)
        nc.sync.dma_start(d10_d[:], d10_s[:])

        # block-diagonal D tiles: dblk [128=(par,n), 9*128=(k, par'*64+f)]
        # nonzero only where par' == par.
        dblk = cpool.tile([P, NTHR * P], F32R)
        zrow = cpool.tile([1, EMBED_DIM], F32)
        nc.vector.memset(zrow[:], 0.0)
        zero_d = dpool.tile([1, EMBED_DIM], F32)
        nc.sync.dma_start(zero_d[:], zrow[:])
        dsrc = (
            d10_d[0:NTHR, :]          # [9, 64]
            .unsqueeze(0)             # [1, 9, 64]
            .broadcast_to([64, NTHR, EMBED_DIM])
        )
        zsrc = (
            zero_d[:]
            .squeeze(0)               # [64]
            .unsqueeze(0).unsqueeze(0)  # [1, 1, 64]
            .broadcast_to([64, NTHR, EMBED_DIM])
        )
        for h in range(2):
            half = dblk[h * 64 : (h + 1) * 64, :].rearrange(
                "n (k g f) -> n k g f", k=NTHR, g=2
            )
            nc.gpsimd.dma_start(half[:, :, h, :], dsrc)
            nc.gpsimd.dma_start(half[:, :, 1 - h, :], zsrc)

        # base bias per (par, f) partition: 64*(EW[0]+b)
        base_col = cpool.tile([P, 1], F32)
        brow = d10_d[NTHR : NTHR + 1, :].squeeze(0).unsqueeze(1)  # [64, 1]
        for h in range(2):
            nc.gpsimd.dma_start(base_col[h * 64 : (h + 1) * 64, :], brow)

        # ---------------- P1: load x, transpose, local min/max ----------------
        x_nat = cpool.tile([P, T_ALL * IN_DIM], F32)
        x_view = x_d.rearrange("(p t) n -> p (t n)", p=P)
        macc_min = cpool.tile([P, T_CHUNK * IN_DIM], F32)
        macc_max = cpool.tile([P, T_CHUNK * IN_DIM], F32)
        u_t = cpool.tile([P, UCOLS], F32)

        csz = T_CHUNK * IN_DIM  # 512
        for c in range(N_CHUNKS):
            sl = slice(c * csz, (c + 1) * csz)
            nc.sync.dma_start(x_nat[:, sl], x_view[:, sl])
            if c == 0:
                nc.vector.tensor_copy(macc_min[:], x_nat[:, sl])
                nc.gpsimd.tensor_copy(macc_max[:], x_nat[:, sl])
            else:
                nc.vector.tensor_tensor(macc_min[:], macc_min[:], x_nat[:, sl], ALU.min)
                nc.vector.tensor_tensor(macc_max[:], macc_max[:], x_nat[:, sl], ALU.max)
            # 4 transposes into one PSUM tile, one wide ACT eviction
            ps_tp = tp_psum.tile([P, 4 * P], F32, tag="tp")
            for jj in range(4):
                j = c * 4 + jj
                nc.tensor.transpose(
                    ps_tp[:, jj * P : (jj + 1) * P],
                    x_nat[:, j * P : (j + 1) * P],
                    ident[:],
                )
            nc.scalar.activation(
                u_t[:, c * 4 * P : (c + 1) * 4 * P], ps_tp[:], ACT.Copy
            )

        # fold t' (8) then partitions -> per-column stats
        stat128 = cpool.tile([P, P], F32)
        nc.vector.tensor_reduce(
            stat128[:, 0:64],
            macc_min[:].rearrange("p (t n) -> p n t", t=T_CHUNK),
            AX.X, ALU.min,
        )
        nc.vector.tensor_reduce(
            stat128[:, 64:P],
            macc_max[:].rearrange("p (t n) -> p n t", t=T_CHUNK),
            AX.X, ALU.max,
        )
        ps_st = mc_psum.tile([P, P], F32, tag="mc")
        nc.tensor.transpose(ps_st[:], stat128[:], ident[:])
        lstat = cpool.tile([P, 1], F32)
        nc.vector.tensor_reduce(lstat[0:64, :], ps_st[0:64, :], AX.X, ALU.min)
        nc.vector.tensor_reduce(lstat[64:P, :], ps_st[64:P, :], AX.X, ALU.max)
        # negate the max half so the post-AG reduce is a single min
        nc.vector.tensor_scalar(lstat[64:P, :], lstat[64:P, :], -1.0, None, ALU.mult)

        # ---------------- P2: AllGather + global stats ----------------
        cc_in = dpool.tile([1, P], F32)
        nc.sync.dma_start(cc_in[:], lstat[:])
        cc_out = dpool.tile([N_CORES, P], F32, addr_space="Shared")
        nc.gpsimd.collective_compute(
            "AllGather",
            ALU.bypass,
            replica_groups=[list(range(N_CORES))],
            ins=[cc_in[:]],
            outs=[cc_out[:]],
        )
        # gather to SBUF contiguously, then PE-transpose the (min | negmax)
        # column blocks into BOTH partition halves so every (par, n) partition
        # sees min[n] and negmax[n] as per-partition scalars.
        cc_sb = cpool.tile([N_CORES, P], F32)
        nc.sync.dma_start(cc_sb[:], cc_out[:])
        # regular matmul with duplicated weights: out[(par,n), r] = cc_sb[r, n]
        ps_mn = mc_psum.tile([P, N_CORES], F32, tag="mc")
        ps_mx = mc_psum.tile([P, N_CORES], F32, tag="mc")
        for s, ps in ((0, ps_mn), (1, ps_mx)):
            ccw = cpool.tile([N_CORES, P], F32, name=f"ccw_{s}")
            for h in range(2):
                nc.vector.tensor_copy(
                    ccw[:, h * 64 : (h + 1) * 64], cc_sb[:, s * 64 : (s + 1) * 64]
                )
            nc.tensor.matmul(
                ps[:], ccw[:], ident[0:N_CORES, 0:N_CORES], start=True, stop=True
            )
        gmin = cpool.tile([P, 1], F32)
        nc.vector.tensor_reduce(gmin[:], ps_mn[:], AX.X, ALU.min)
        gnegmax = cpool.tile([P, 1], F32)
        nc.vector.tensor_reduce(gnegmax[:], ps_mx[:], AX.X, ALU.min)
        # range+eps = (-negmax + eps) - min ; per (par, n) partition
        rtmp = cpool.tile([P, 1], F32)
        nc.vector.tensor_scalar(rtmp[:], gnegmax[:], -1.0, EPS, ALU.mult, ALU.add)
        range_dup = cpool.tile([P, 1], F32)
        nc.vector.tensor_scalar(range_dup[:], rtmp[:], gmin[:, 0:1], None, ALU.subtract)
        # thresholds: s_thr[(par,n), k] = min[n] + bins[k]*(range[n]+eps)
        s_thr = cpool.tile([P, NUM_BINS], F32)
        nc.vector.tensor_scalar(
            s_thr[:], bins_bc[:], range_dup[:, 0:1], gmin[:, 0:1],
            ALU.mult, ALU.add,
        )

        # ---------------- P3: indicators + matmul + store ----------------
        n_sub = UCOLS // MM_N  # 4
        ps_out = [out_psum.tile([P, MM_N], F32, tag="out", name=f"pso_{s}")
                  for s in range(n_sub)]
        inds = []
        for k in range(NTHR):
            ind = ipool.tile([P, UCOLS], F32R, tag="ind", name=f"ind_{k}")
            nc.vector.tensor_scalar(
                ind[:], u_t[:], s_thr[:, k : k + 1], None, ALU.is_gt
            )
            inds.append(ind)
            for sub in range(n_sub):
                nc.tensor.matmul(
                    ps_out[sub][:],
                    dblk[:, k * P : (k + 1) * P],
                    ind[:, sub * MM_N : (sub + 1) * MM_N],
                    start=(k == 0),
                    stop=(k == NTHR - 1),
                )
        for sub in range(n_sub):
            out_s = opool.tile([P, MM_N], F32, tag="outs", name=f"outs_{sub}")
            nc.scalar.activation(
                out_s[:], ps_out[sub][:], ACT.Identity, bias=base_col[:, 0:1]
            )
            nc.sync.dma_start(
                out_d[:, sub * MM_N : (sub + 1) * MM_N], out_s[:]
            )


_CACHED = {}


def _build(loop=1):
    if loop in _CACHED:
        return _CACHED[loop]
    nc = bacc.Bacc(
        "TRN2",
        target_bir_lowering=False,
        debug=False,
        enable_asserts=True,
        num_devices=N_CORES,
    )
    with tile.TileContext(nc) as tc:
        io = (
            nc.dram_tensor("x_sh", [B_C, IN_DIM], F32, kind="ExternalInput").ap(),
            nc.dram_tensor("bin_logits", [NUM_BINS], F32, kind="ExternalInput").ap(),
            nc.dram_tensor("embed", [NUM_BINS, EMBED_DIM], F32, kind="ExternalInput").ap(),
            nc.dram_tensor("W", [EMBED_DIM, EMBED_DIM], F32, kind="ExternalInput").ap(),
            nc.dram_tensor("b", [EMBED_DIM], F32, kind="ExternalInput").ap(),
            nc.dram_tensor("out_t", [P, UCOLS], F32, kind="ExternalOutput").ap(),
        )
        for it in range(loop):
            _trace_kernel(tc, io, tag=f"_{it}" if loop > 1 else "")
    nc.compile()
    _CACHED[loop] = nc
    return nc


def _make_in_maps(x, bin_logits, embed, W, b):
    maps = []
    for c in range(N_CORES):
        maps.append(
            {
                "x_sh": np.ascontiguousarray(x[c * B_C : (c + 1) * B_C]),
                "bin_logits": np.asarray(bin_logits),
                "embed": np.asarray(embed),
                "W": np.asarray(W),
                "b": np.asarray(b),
            }
        )
    return maps


def _unshard(results):
    shards = []
    for c in range(N_CORES):
        out_t = results[c]["out_t"]  # [128=(par,f), 2048=(j,p)]
        shard = (
            out_t.reshape(2, EMBED_DIM, T_ALL // 2, P)
            .transpose(3, 2, 0, 1)           # [p, j, par, f]
            .reshape(B_C, EMBED_DIM)         # b = p*32 + j*2 + par
        )
        shards.append(shard)
    return np.ascontiguousarray(np.concatenate(shards, axis=0))


def kernel(x, bin_logits, embed, W, b):
    nc = _build()
    in_maps = _make_in_maps(np.asarray(x, dtype=np.float32), bin_logits, embed, W, b)
    res = bass_utils.run_bass_kernel_spmd(nc, in_maps, core_ids=list(range(N_CORES)))
    return _unshard(res.results)


# revision 29
# speedup vs baseline: 164.0639x; 164.0639x over previous
"""DyBEM layer (histogram binning + embedding sum + linear) on 8 trn2 cores.

Math reduction
--------------
ref: xmin/xmax per column n over the batch; u = (x-xmin)/(xmax-xmin+eps);
     bins = cumsum(softmax(bin_logits)); idx = clip(searchsorted(bins, u), 0, 9)
     out  = einsum('bne,fe->bf', embed[idx], W) + IN_DIM * b

Let EW = embed @ W.T (shape [10,64]).  With g[b,k] = #{n : u[b,n] > bins[k]}
(k = 0..8):

  out[b] = IN_DIM*(EW[0] + b) + sum_k g[b,k] * (EW[k+1] - EW[k])

and u[b,n] > bins[k]  <=>  x[b,n] > T[k,n] := xmin[n] + bins[k]*(range[n]+eps).

Per core (batch shard of 4096 rows): local per-column min/max, AllGather the
8 cores' (min, -max) vectors, local reduce, then 9 indicator passes feeding a
PE contraction with the tiny D[k] = EW[k+1]-EW[k] rows.  The base row
IN_DIM*(EW[0]+b) is added as a per-partition bias during PSUM eviction.

Layout: x loads natural as [128, 32*64] (8KB contiguous per partition; SBUF
partition p holds batch rows p*32+t).  Each [128,128] column pair (t=2j,2j+1)
is PE-transposed into u [128=(par,n), 16*128] where column j*128+p holds batch
row b = p*32 + 2j + par split by t-parity across partition halves.  A single
per-partition-scalar is_gt (DVE 2x mode) then evaluates one bin for all 4096
rows, and matmuls with block-diagonal D tiles [128=(par,n), 128=(par,f)] keep
the parities separate, producing out.T [128=(par,f), (j,p)] in PSUM.  The host
undoes the (p,j,par) column permutation after gathering.
"""

import numpy as np

import concourse.bass as bass
import concourse.mybir as mybir
import concourse.tile as tile
from concourse import bacc, bass_utils
from concourse.masks import make_identity

F32 = mybir.dt.float32
F32R = mybir.dt.float32r
ALU = mybir.AluOpType
AX = mybir.AxisListType
ACT = mybir.ActivationFunctionType

B_FULL, IN_DIM, NUM_BINS, EMBED_DIM = 32768, 64, 10, 64
N_CORES = 8
B_C = B_FULL // N_CORES          # 4096 rows per core
EPS = 1e-6
P = 128
T_ALL = B_C // P                 # 32 row-groups (t index)
N_CHUNKS = 8                     # x load chunks
T_CHUNK = T_ALL // N_CHUNKS      # 8 t's per chunk
NTHR = NUM_BINS - 1              # 9 real thresholds
UCOLS = B_C // 2                 # 2048 u columns (2 rows per column)
MM_N = 512                       # matmul moving free size (one PSUM bank)


def _trace_kernel(tc, io, tag=""):
    nc = tc.nc
    x_d, bl_d, emb_d, w_d, b_d, out_d = io

    with (
        tc.tile_pool(name=f"const{tag}", bufs=1) as cpool,
        tc.tile_pool(name=f"ind{tag}", bufs=9) as ipool,
        tc.tile_pool(name=f"outs{tag}", bufs=4) as opool,
        tc.tile_pool(name=f"tp_psum{tag}", bufs=2, space="PSUM") as tp_psum,
        tc.tile_pool(name=f"out_psum{tag}", bufs=4, space="PSUM") as out_psum,
        tc.tile_pool(name=f"mc_psum{tag}", bufs=2, space="PSUM") as mc_psum,
        tc.tile_pool(name=f"dram{tag}", bufs=1, space="DRAM") as dpool,
    ):
        # ---------------- P0: parameters (independent of x) ----------------
        ident = cpool.tile([P, P], F32)
        make_identity(nc, ident[:])

        # bins = cumsum(softmax(bin_logits))
        bl_row = cpool.tile([1, NUM_BINS], F32)
        nc.gpsimd.dma_start(bl_row[:], bl_d.unsqueeze(0))
        e_row = cpool.tile([1, NUM_BINS], F32)
        nc.scalar.activation(e_row[:], bl_row[:], ACT.Exp)
        ssum = cpool.tile([1, 1], F32)
        nc.vector.tensor_reduce(ssum[:], e_row[:], AX.X, ALU.add)
        rsum = cpool.tile([1, 1], F32)
        nc.vector.reciprocal(rsum[:], ssum[:])
        prob_a = cpool.tile([1, NUM_BINS], F32)
        nc.vector.tensor_scalar(prob_a[:], e_row[:], rsum[:, 0:1], None, ALU.mult)
        prob_b = cpool.tile([1, NUM_BINS], F32)
        cur, nxt = prob_a, prob_b
        for sh in (1, 2, 4, 8):
            nc.vector.tensor_copy(nxt[:, 0:sh], cur[:, 0:sh])
            nc.vector.tensor_tensor(
                nxt[:, sh:NUM_BINS], cur[:, sh:NUM_BINS], cur[:, 0 : NUM_BINS - sh],
                ALU.add,
            )
            cur, nxt = nxt, cur
        # bins broadcast to every partition: [128, 10] via DRAM bounce
        bins_d = dpool.tile([1, NUM_BINS], F32)
        nc.gpsimd.dma_start(bins_d[:], cur[:])
        bins_bc = cpool.tile([P, NUM_BINS], F32)
        nc.gpsimd.dma_start(
            bins_bc[:],
            bins_d[:].broadcast_to([P, NUM_BINS]),
        )

        # EW = embed @ W.T ; D10 rows 0..8 = EW[k+1]-EW[k], row 9 = 64*(EW[0]+b)
        emb_s = cpool.tile([NUM_BINS, EMBED_DIM], F32)
        nc.gpsimd.dma_start(emb_s[:], emb_d)
        w_s = cpool.tile([EMBED_DIM, EMBED_DIM], F32)
        nc.gpsimd.dma_start(w_s[:], w_d)

        ps_embT = mc_psum.tile([EMBED_DIM, NUM_BINS], F32, tag="mc")
        nc.tensor.transpose(ps_embT[:], emb_s[:], ident[0:NUM_BINS, 0:NUM_BINS])
        embT_s = cpool.tile([EMBED_DIM, NUM_BINS], F32)
        nc.scalar.activation(embT_s[:], ps_embT[:], ACT.Copy)

        ps_wt = mc_psum.tile([EMBED_DIM, EMBED_DIM], F32, tag="mc")
        nc.tensor.transpose(ps_wt[:], w_s[:], ident[0:EMBED_DIM, 0:EMBED_DIM])
        wt_s = cpool.tile([EMBED_DIM, EMBED_DIM], F32)
        nc.scalar.activation(wt_s[:], ps_wt[:], ACT.Copy)

        ps_ew = mc_psum.tile([NUM_BINS, EMBED_DIM], F32, tag="mc")
        nc.tensor.matmul(ps_ew[:], embT_s[:], wt_s[:], start=True, stop=True)
        ew_ext = cpool.tile([NUM_BINS + 1, EMBED_DIM], F32)
        nc.scalar.activation(ew_ext[0:NUM_BINS, :], ps_ew[:], ACT.Copy)
        nc.gpsimd.dma_start(ew_ext[NUM_BINS : NUM_BINS + 1, :], b_d.unsqueeze(0))

        # MT [11, 10]: D10 = MT.T @ ew_ext (bidiagonal diffs; col 9 = 64*(row0+rowb))
        mt = cpool.tile([NUM_BINS + 1, NUM_BINS], F32)
        nc.gpsimd.memset(mt[:], 0.0)
        nc.gpsimd.affine_select(
            out=mt[:, 0:NTHR], in_=mt[:, 0:NTHR], compare_op=ALU.not_equal,
            fill=-1.0, base=0, pattern=[[-1, NTHR]], channel_multiplier=1,
        )
        nc.gpsimd.affine_select(
            out=mt[:, 0:NTHR], in_=mt[:, 0:NTHR], compare_op=ALU.not_equal,
            fill=1.0, base=-1, pattern=[[-1, NTHR]], channel_multiplier=1,
        )
        nc.gpsimd.affine_select(
            out=mt[:, NTHR : NTHR + 1], in_=mt[:, NTHR : NTHR + 1],
            compare_op=ALU.not_equal, fill=float(IN_DIM),
            base=0, pattern=[[-1, 1]], channel_multiplier=1,
        )
        nc.gpsimd.affine_select(
            out=mt[:, NTHR : NTHR + 1], in_=mt[:, NTHR : NTHR + 1],
            compare_op=ALU.not_equal, fill=float(IN_DIM),
            base=-NUM_BINS, pattern=[[-1, 1]], channel_multiplier=1,
        )
        ps_d10 = mc_psum.tile([NUM_BINS, EMBED_DIM], F32, tag="mc")
        nc.tensor.matmul(ps_d10[:], mt[:], ew_ext[:], start=True, stop=True)
        d10_s = cpool.tile([NUM_BINS, EMBED_DIM], F32)
        nc.scalar.activation(d10_s[:], ps_d10[:], ACT.Copy)
        d10_d = dpool.tile([NUM_BINS, EMBED_DIM], F32)
        nc.gpsimd.dma_start(d10_d[:], d10_s[:])

        # block-diagonal D tiles: dblk [128=(par,n), 9*128=(k, par'*64+f)],
        # nonzero only where par' == par.  Built from a [2, 1152] master row
        # pair and one contiguous SBUF->SBUF partition-broadcast DMA.
        dblk = cpool.tile([P, NTHR * P], F32R)
        mini = cpool.tile([2, NTHR * P], F32)
        nc.gpsimd.memset(mini[:], 0.0)
        for h in range(2):
            mrow = mini[h : h + 1, :].rearrange("o (k g f) -> o k g f", k=NTHR, g=2)
            nc.gpsimd.dma_start(mrow[:, :, h, :], d10_d[0:NTHR, :].unsqueeze(0))
        nc.gpsimd.dma_start(
            dblk[:],
            mini[:].unsqueeze(1).broadcast_to([2, 64, NTHR * P]).bitcast(F32R),
        )

        # base bias per (par, f) partition: 64*(EW[0]+b)
        base_col = cpool.tile([P, 1], F32)
        brow = d10_d[NTHR : NTHR + 1, :].squeeze(0).unsqueeze(1)  # [64, 1]
        for h in range(2):
            nc.gpsimd.dma_start(base_col[h * 64 : (h + 1) * 64, :], brow)

        # ---------------- P1: load x, transpose, local min/max ----------------
        x_nat = cpool.tile([P, T_ALL * IN_DIM], F32)
        x_view = x_d.rearrange("(p t) n -> p (t n)", p=P)
        macc_min = cpool.tile([P, T_CHUNK * IN_DIM], F32)
        macc_max = cpool.tile([P, T_CHUNK * IN_DIM], F32)
        u_t = cpool.tile([P, UCOLS], F32)

        csz = T_CHUNK * IN_DIM  # 256 with N_CHUNKS=8
        for c in range(N_CHUNKS):
            sl = slice(c * csz, (c + 1) * csz)
            (nc.sync if c % 2 == 0 else nc.scalar).dma_start(x_nat[:, sl], x_view[:, sl])
            if c == 0:
                nc.vector.tensor_copy(macc_min[:], x_nat[:, sl])
                nc.gpsimd.tensor_copy(macc_max[:], x_nat[:, sl])
            else:
                nc.vector.tensor_tensor(macc_min[:], macc_min[:], x_nat[:, sl], ALU.min)
                nc.vector.tensor_tensor(macc_max[:], macc_max[:], x_nat[:, sl], ALU.max)
            # 4 transposes into one PSUM tile (spanning 2 load chunks), then
            # one wide ACT eviction
            if c % 2 == 1:
                g = c // 2
                ps_tp = tp_psum.tile([P, 4 * P], F32, tag="tp")
                for jj in range(4):
                    j = g * 4 + jj
                    nc.tensor.transpose(
                        ps_tp[:, jj * P : (jj + 1) * P],
                        x_nat[:, j * P : (j + 1) * P],
                        ident[:],
                    )
                nc.scalar.activation(
                    u_t[:, g * 4 * P : (g + 1) * 4 * P], ps_tp[:], ACT.Copy
                )

        # fold t' (8) then partitions -> per-column stats
        stat128 = cpool.tile([P, P], F32)
        nc.vector.tensor_reduce(
            stat128[:, 0:64],
            macc_min[:].rearrange("p (t n) -> p n t", t=T_CHUNK),
            AX.X, ALU.min,
        )
        nc.vector.tensor_reduce(
            stat128[:, 64:P],
            macc_max[:].rearrange("p (t n) -> p n t", t=T_CHUNK),
            AX.X, ALU.max,
        )
        ps_st = mc_psum.tile([P, P], F32, tag="mc")
        nc.tensor.transpose(ps_st[:], stat128[:], ident[:])
        lstat = cpool.tile([P, 1], F32)
        nc.vector.tensor_reduce(lstat[0:64, :], ps_st[0:64, :], AX.X, ALU.min)
        nc.vector.tensor_reduce(lstat[64:P, :], ps_st[64:P, :], AX.X, ALU.max)
        # negate the max half (and fold in -eps) so the post-AG reduce is a
        # single min: stored value = -(max + eps)
        nc.vector.tensor_scalar(
            lstat[64:P, :], lstat[64:P, :], -1.0, -EPS, ALU.mult, ALU.add
        )

        # ---------------- P2: AllGather + global stats ----------------
        cc_in = dpool.tile([1, P], F32)
        nc.gpsimd.dma_start(cc_in[:], lstat[:])
        cc_out = dpool.tile([N_CORES, P], F32, addr_space="Shared")
        nc.gpsimd.collective_compute(
            "AllGather",
            ALU.bypass,
            replica_groups=[list(range(N_CORES))],
            ins=[cc_in[:]],
            outs=[cc_out[:]],
        )
        # gather (min | negmax) straight into per-partition layout with 4
        # small strided DMAs: post2[(par,n), (s,r)] = cc_out[r, s*64+n]
        post2 = cpool.tile([P, 2 * N_CORES], F32)
        qs = (nc.gpsimd, nc.gpsimd, nc.gpsimd, nc.gpsimd)
        for s in range(2):
            for h in range(2):
                src = cc_out[:, s * 64 : (s + 1) * 64].transpose([1, 0])
                qs[s * 2 + h].dma_start(
                    post2[h * 64 : (h + 1) * 64, s * N_CORES : (s + 1) * N_CORES],
                    src,
                )
        pr = cpool.tile([P, 2], F32)
        nc.vector.tensor_reduce(
            pr[:], post2[:].rearrange("p (s r) -> p s r", s=2), AX.X, ALU.min
        )
        # range+eps = (-gnegmax) - min  (eps already folded into negmax half)
        range_dup = cpool.tile([P, 1], F32)
        nc.vector.tensor_scalar(
            range_dup[:], pr[:, 1:2], -1.0, pr[:, 0:1], ALU.mult, ALU.subtract
        )
        # thresholds: s_thr[(par,n), k] = min[n] + bins[k]*(range[n]+eps)
        s_thr = cpool.tile([P, NUM_BINS], F32)
        nc.vector.tensor_scalar(
            s_thr[:], bins_bc[:], range_dup[:, 0:1], pr[:, 0:1],
            ALU.mult, ALU.add,
        )

        # ---------------- P3: indicators + matmul + store ----------------
        n_sub = UCOLS // MM_N  # 4
        ps_out = [out_psum.tile([P, MM_N], F32, tag="out", name=f"pso_{s}")
                  for s in range(n_sub)]
        inds = []
        for k in range(NTHR):
            ind = ipool.tile([P, UCOLS], F32R, tag="ind", name=f"ind_{k}")
            eng = nc.gpsimd if k % 3 == 1 else nc.vector
            eng.tensor_scalar(
                ind[:], u_t[:], s_thr[:, k : k + 1], None, ALU.is_gt
            )
            inds.append(ind)
            for sub in range(n_sub):
                nc.tensor.matmul(
                    ps_out[sub][:],
                    dblk[:, k * P : (k + 1) * P],
                    ind[:, sub * MM_N : (sub + 1) * MM_N],
                    start=(k == 0),
                    stop=(k == NTHR - 1),
                )
        for sub in range(n_sub):
            out_s = opool.tile([P, MM_N], F32, tag="outs", name=f"outs_{sub}")
            if sub % 2 == 0:
                nc.scalar.activation(
                    out_s[:], ps_out[sub][:], ACT.Identity, bias=base_col[:, 0:1]
                )
            else:
                nc.vector.tensor_scalar(
                    out_s[:], ps_out[sub][:], base_col[:, 0:1], None, ALU.add
                )
            (nc.sync if sub % 2 == 0 else nc.scalar).dma_start(
                out_d[:, sub * MM_N : (sub + 1) * MM_N], out_s[:]
            )


_CACHED = {}


def _build(loop=1):
    if loop in _CACHED:
        return _CACHED[loop]
    nc = bacc.Bacc(
        "TRN2",
        target_bir_lowering=False,
        debug=False,
        enable_asserts=True,
        num_devices=N_CORES,
    )
    with tile.TileContext(nc) as tc:
        io = (
            nc.dram_tensor("x_sh", [B_C, IN_DIM], F32, kind="ExternalInput").ap(),
            nc.dram_tensor("bin_logits", [NUM_BINS], F32, kind="ExternalInput").ap(),
            nc.dram_tensor("embed", [NUM_BINS, EMBED_DIM], F32, kind="ExternalInput").ap(),
            nc.dram_tensor("W", [EMBED_DIM, EMBED_DIM], F32, kind="ExternalInput").ap(),
            nc.dram_tensor("b", [EMBED_DIM], F32, kind="ExternalInput").ap(),
            nc.dram_tensor("out_t", [P, UCOLS], F32, kind="ExternalOutput").ap(),
        )
        for it in range(loop):
            _trace_kernel(tc, io, tag=f"_{it}" if loop > 1 else "")
    nc.compile()
    _CACHED[loop] = nc
    return nc


def _make_in_maps(x, bin_logits, embed, W, b):
    maps = []
    for c in range(N_CORES):
        maps.append(
            {
                "x_sh": np.ascontiguousarray(x[c * B_C : (c + 1) * B_C]),
                "bin_logits": np.asarray(bin_logits),
                "embed": np.asarray(embed),
                "W": np.asarray(W),
                "b": np.asarray(b),
            }
        )
    return maps


def _unshard(results):
    shards = []
    for c in range(N_CORES):
        out_t = results[c]["out_t"]  # [128=(par,f), 2048=(j,p)]
        shard = (
            out_t.reshape(2, EMBED_DIM, T_ALL // 2, P)
            .transpose(3, 2, 0, 1)           # [p, j, par, f]
            .reshape(B_C, EMBED_DIM)         # b = p*32 + j*2 + par
        )
        shards.append(shard)
    return np.ascontiguousarray(np.concatenate(shards, axis=0))


def kernel(x, bin_logits, embed, W, b):
    nc = _build()
    in_maps = _make_in_maps(np.asarray(x, dtype=np.float32), bin_logits, embed, W, b)
    res = bass_utils.run_bass_kernel_spmd(nc, in_maps, core_ids=list(range(N_CORES)))
    return _unshard(res.results)
